# revision 26
# baseline (speedup 1.0000x reference)
"""Trainium2 Bass kernel for factorized space-time attention.

Computation (per batch b of 8, one NeuronCore each):
  qkv = x @ w_qkv.T                      (3136, 2304)
  heads 0-5:  spatial attention over 196 patches within each of 16 frames
  heads 6-11: temporal attention over groups of 16 consecutive tokens
              (raw-reshape semantics of the reference)
  out = concat(head outputs) @ w_proj.T + b_proj

Strategy: data-parallel over batch (8 cores). All activations kept
feature-major ([d, n]) on chip so every matmul contraction runs over the
partition dim with no on-device transposes; x / weights are pre-transposed
host-side. V is produced token-major directly by flipping the projection
matmul orientation. Temporal attention runs on 112x112 score windows with
a block-diagonal mask (7 x 16x16). Softmax skips max-subtraction (scores
~N(0,1)) and normalizes via a ones-matmul partition-broadcast of 1/rowsum
(the denominator comes free from a ones column appended to V).

The program is software-pipelined: attention of superblock s (SB = 784
tokens = lcm(196,16)) interleaves with the QKV/V projections of SB s+1
and the output projection of SB s-1 in emission (= engine-queue) order,
so the PE has independent matmul work during the scalar/vector stages of
the attention chain. The in-loop projection of SB (s+1)%4 at s=3 primes
SB0 for the NEXT loop iteration (tile tag rotation lands it on the same
SBUF addresses the prologue used). Attention AV results are copied out of
PSUM to bf16 SBUF immediately so the PSUM bank frees early (deeper
pipelining) and the normalize muls run in the DVE's fast all-SBUF 2-byte
mode. psum->SBUF copies alternate scalar/vector engines to balance load.

NOTE an undocumented erratum: a second matmul writing a column-offset
region of an in-use PSUM bank fails at runtime for some shapes (observed
with M<=96, operand partition base 64). The spatial score merge
([128,392], offsets 0/196, M=128/68) is validated on HW; the temporal
variant ([112,224], offset 112) is NOT safe and temporal scores therefore
use separate [112,112] tiles.
"""

import sys

# concourse normally comes from the axon site tree (sitecustomize); the
# append is a fallback so a bare environment still finds it.
if "/opt/trn_rl_repo" not in sys.path:
    sys.path.append("/opt/trn_rl_repo")

import contextlib

import numpy as np

import concourse.bass as bass  # noqa: F401  (engine namespaces live on nc)
import concourse.mybir as mybir
import concourse.tile as tile
from concourse import bacc
from concourse.bass_utils import run_bass_kernel_spmd

F32 = mybir.dt.float32
BF16 = mybir.dt.bfloat16
AF = mybir.ActivationFunctionType

# problem dims (hardcoded per contract)
B = 8
F = 16
P = 196
D = 768
NH = 12
HD = 64
N = F * P  # 3136
E3 = 3 * D  # 2304
SB = 784  # superblock = lcm(196, 16) tokens
NSB = N // SB  # 4
FPSB = SB // P  # 4 frames per superblock
WPSB = SB // 112  # 7 temporal windows per superblock
SCALE = HD ** -0.5

# compute dtype for matmul inputs ("f32" safest, "bf16" 4x faster on PE:
# fp32 matmuls stream at 4 cycles/row vs 1 for bf16)
COMPUTE = "bf16"

# software-pipeline the three streams (debug switch)
PIPE = True

_CACHE = {}


def _build(compute: str, reps: int = 1):
    """Build + bass-compile the per-core kernel. Returns the Bacc object."""
    cdt = BF16 if compute == "bf16" else F32
    F32R = mybir.dt.float32r

    def mmcast(ap):
        return ap.bitcast(F32R) if compute == "f32r" else ap

    nc = bacc.Bacc("TRN2", target_bir_lowering=False, debug=False, num_devices=B)

    xt_d = nc.dram_tensor("xt", (D, N), cdt, kind="ExternalInput")
    wqkv_d = nc.dram_tensor("wqkvT", (D, E3), cdt, kind="ExternalInput")
    wproj_d = nc.dram_tensor("wprojT", (D, D), cdt, kind="ExternalInput")
    bias_d = nc.dram_tensor("bias", (D, 1), F32, kind="ExternalInput")
    mask_d = nc.dram_tensor("mask", (112, 112), cdt, kind="ExternalInput")
    out_d = nc.dram_tensor("outT", (D, N), F32, kind="ExternalOutput")

    with tile.TileContext(nc) as tc:
        with (
            tc.tile_pool(name="const", bufs=1) as cpool,
            tc.tile_pool(name="work", bufs=1) as wpool,
            tc.tile_pool(name="small", bufs=4) as spool,
            tc.tile_pool(name="psum", bufs=2, space="PSUM") as ppool,
        ):
            # ---- constants -------------------------------------------------
            wq = []
            for dc in range(6):
                t = cpool.tile([128, E3], cdt, tag=f"wq{dc}", name=f"wq{dc}")
                nc.sync.dma_start(t[:], wqkv_d.ap()[128 * dc : 128 * (dc + 1), :])
                wq.append(t)
            wp = []
            for dc in range(6):
                t = cpool.tile([128, D], cdt, tag=f"wp{dc}", name=f"wp{dc}")
                nc.sync.dma_start(t[:], wproj_d.ap()[128 * dc : 128 * (dc + 1), :])
                wp.append(t)
            bias_t = cpool.tile([128, 6], F32, tag="bias", name="bias_t")
            nc.sync.dma_start(
                bias_t[:], bias_d.ap().rearrange("(e p) one -> p (e one)", p=128)
            )
            mask2_t = cpool.tile([112, 224], cdt, tag="mask", name="mask2_t")
            nc.sync.dma_start(mask2_t[:, 0:112], mask_d.ap())
            nc.sync.dma_start(mask2_t[:, 112:224], mask_d.ap())
            zeros_col = cpool.tile([128, 1], F32, tag="zeros_c", name="zeros_col")
            nc.gpsimd.memset(zeros_col[:], 0.0)
            # row 64 of ones (the psum row the softmax sums land on) is the
            # stationary operand of the 1/sum partition-broadcast matmul
            ones64 = cpool.tile([65, 64], cdt, tag="ones64", name="ones64")
            nc.gpsimd.memset(ones64[:], 1.0)

            # flexible-engine copy: alternate scalar / vector to balance load
            flex_ctr = [0]

            def flex_copy(dst, src):
                flex_ctr[0] += 1
                if flex_ctr[0] % 2 == 0:
                    nc.scalar.copy(dst, src)
                else:
                    nc.vector.tensor_copy(dst, src)

            # per-superblock tile sets, filled by the P stream
            sbt = {}

            def p_chunks(s):
                """Projection stream for superblock s: xts DMA, QK proj into
                qkvt (feature-major), V proj token-major. Returns closures."""
                so = SB * s
                st = sbt[s] = {}
                chunks = []

                def dma_x():
                    xts = st["xts"] = []
                    for dc in range(6):
                        t = wpool.tile([128, SB], cdt, tag=f"xts{dc}", bufs=2,
                                       name=f"xts{dc}_{s}")
                        nc.sync.dma_start(
                            t[:], xt_d.ap()[128 * dc : 128 * (dc + 1), so : so + SB]
                        )
                        xts.append(t)
                    st["qkvt"] = [None] * 12
                chunks.append(dma_x)

                def qk_chunk(ti, j):
                    def run():
                        if st["qkvt"][ti] is None:
                            st["qkvt"][ti] = wpool.tile(
                                [128, SB], cdt, tag=f"qkvt{ti}", bufs=2,
                                name=f"qkvt{ti}_{s}")
                        qt = st["qkvt"][ti]
                        ps = ppool.tile([128, 392], F32, tag="mm", bufs=2,
                                        name=f"ps_qk{s}_{ti}_{j}")
                        for dc in range(6):
                            nc.tensor.matmul(
                                ps[:],
                                mmcast(wq[dc][:, 128 * ti : 128 * (ti + 1)]),
                                mmcast(st["xts"][dc][:, 392 * j : 392 * (j + 1)]),
                                start=(dc == 0),
                                stop=(dc == 5),
                            )
                        flex_copy(qt[:, 392 * j : 392 * (j + 1)], ps[:])
                    return run

                def v_chunk(msz, tok0, wcol0, key, idx):
                    def run():
                        vt_ = wpool.tile([msz, 390], cdt, tag=f"{key}{idx}",
                                         bufs=2, name=f"{key}{idx}_{s}")
                        st[key][idx] = vt_
                        ps = ppool.tile([msz, 384], F32, tag="mm", bufs=2,
                                        name=f"ps_{key}{s}_{idx}")
                        for dc in range(6):
                            nc.tensor.matmul(
                                ps[:],
                                mmcast(st["xts"][dc][:, tok0 : tok0 + msz]),
                                mmcast(wq[dc][:, wcol0 : wcol0 + 384]),
                                start=(dc == 0),
                                stop=(dc == 5),
                            )
                        flex_copy(
                            vt_.rearrange("p (h c) -> p h c", c=65)[:, :, 0:64],
                            ps.rearrange("p (h c) -> p h c", c=64),
                        )
                        nc.gpsimd.memset(
                            vt_.rearrange("p (h c) -> p h c", c=65)[:, :, 64:65], 1.0
                        )
                    return run

                st["vs"] = [None] * 8
                st["vt"] = [None] * 7
                # order: QK tiles needed by spatial attention first, then
                # spatial V, then temporal QK, then temporal V
                for ti in (0, 1, 2, 6, 7, 8):
                    for j in range(2):
                        chunks.append(qk_chunk(ti, j))
                for f in range(FPSB):
                    for ci, (m0, msz) in enumerate(((0, 128), (128, 68))):
                        chunks.append(
                            v_chunk(msz, 196 * f + m0, 1536, "vs", 2 * f + ci))
                for ti in (3, 4, 5, 9, 10, 11):
                    for j in range(2):
                        chunks.append(qk_chunk(ti, j))
                for w in range(WPSB):
                    chunks.append(v_chunk(112, 112 * w, 1920, "vt", w))
                return chunks

            def a_groups(s):
                """Attention stream for superblock s. Each head-pair group is
                two zip chunks: front (scores+exp) and back (AV, 1/sum,
                broadcast, normalize into attnT)."""
                st = sbt[s]
                groups = []

                def mk_attnT():
                    st["attnT"] = [
                        wpool.tile([128, SB], cdt, tag=f"attnT{i}", bufs=2,
                                   name=f"attnT{i}_{s}")
                        for i in range(6)
                    ]
                groups.append(mk_attnT)

                def spatial(f, hp):
                    es = []

                    def front():
                        fo = 196 * f
                        qt, kt = st["qkvt"][hp], st["qkvt"][6 + hp]
                        for hi in range(2):
                            pb = 64 * hi
                            stp = ppool.tile([128, 392], F32, tag="st", bufs=3,
                                             name=f"st{s}_{f}_{hp}_{hi}")
                            for ci, (m0, msz) in enumerate(((0, 128), (128, 68))):
                                nc.tensor.matmul(
                                    stp[0:msz, 196 * ci : 196 * ci + 196],
                                    kt[pb : pb + 64, fo + m0 : fo + m0 + msz],
                                    qt[pb : pb + 64, fo : fo + 196],
                                    start=True, stop=True,
                                    skip_group_check=(ci == 1),
                                )
                            e = spool.tile([128, 392], cdt, tag="e", bufs=6,
                                           name=f"e{s}_{f}_{hp}_{hi}")
                            # two exps on exactly the written psum regions
                            nc.scalar.activation(
                                e[:, 0:196], stp[:, 0:196], AF.Exp,
                                bias=zeros_col[:], scale=SCALE,
                            )
                            nc.scalar.activation(
                                e[0:68, 196:392], stp[0:68, 196:392], AF.Exp,
                                bias=zeros_col[:68], scale=SCALE,
                            )
                            es.append(e)

                    def back():
                        fo = 196 * f
                        av = ppool.tile([65, 392], F32, tag="av", bufs=3,
                                        name=f"av{s}_{f}_{hp}")
                        for hi in range(2):
                            h = 2 * hp + hi
                            for ci, msz in enumerate((128, 68)):
                                nc.tensor.matmul(
                                    av[:, 196 * hi : 196 * hi + 196],
                                    st["vs"][2 * f + ci][:, 65 * h : 65 * h + 65],
                                    es[hi][0:msz, 196 * ci : 196 * ci + 196],
                                    start=(ci == 0), stop=(ci == 1),
                                    skip_group_check=(hi == 1),
                                )
                        # free the av bank early: bf16 SBUF copy feeds the
                        # normalize muls (all-SBUF 2-byte => fast DVE mode)
                        avc = spool.tile([64, 392], cdt, tag="avc", bufs=6,
                                         name=f"avc{s}_{f}_{hp}")
                        flex_copy(avc[:], av[0:64, :])
                        r = spool.tile([65, 392], cdt, tag="r", bufs=6,
                                       name=f"r{s}_{f}_{hp}")
                        with nc.allow_low_precision(reason="1/softmax-sum"):
                            nc.vector.reciprocal(r[64:65, :], av[64:65, :])
                        bc = ppool.tile([64, 392], F32, tag="st", bufs=3,
                                        name=f"bc{s}_{f}_{hp}")
                        nc.tensor.matmul(
                            bc[:], ones64[64:65, 0:64], r[64:65, :],
                            start=True, stop=True,
                        )
                        rb = spool.tile([64, 392], cdt, tag="rb", bufs=6,
                                        name=f"rb{s}_{f}_{hp}")
                        flex_copy(rb[:], bc[:])
                        at = st["attnT"][hp]
                        nc.vector.tensor_mul(
                            at[0:64, fo : fo + 196], avc[:, 0:196],
                            rb[:, 0:196],
                        )
                        tmp = spool.tile([64, 196], cdt, tag="tmp", bufs=6,
                                         name=f"tm{s}_{f}_{hp}")
                        nc.vector.tensor_mul(tmp[:], avc[:, 196:392],
                                             rb[:, 196:392])
                        nc.sync.dma_start(at[64:128, fo : fo + 196], tmp[:])

                    return front, back

                def temporal(w, hp):
                    ems = []

                    def front():
                        wo = 112 * w
                        qt, kt = st["qkvt"][3 + hp], st["qkvt"][9 + hp]
                        em = spool.tile([112, 224], cdt, tag="e", bufs=6,
                                        name=f"em{s}_{w}_{hp}")
                        ems.append(em)
                        for hi in range(2):
                            pb = 64 * hi
                            # separate [112,112] tiles: the merged [112,224]
                            # two-region write hits the PSUM erratum (see
                            # module docstring)
                            stp = ppool.tile([112, 112], F32, tag="st", bufs=3,
                                             name=f"ts{s}_{w}_{hp}_{hi}")
                            nc.tensor.matmul(
                                stp[:],
                                kt[pb : pb + 64, wo : wo + 112],
                                qt[pb : pb + 64, wo : wo + 112],
                                start=True, stop=True,
                            )
                            e = spool.tile([112, 112], cdt, tag="e", bufs=6,
                                           name=f"et{s}_{w}_{hp}_{hi}")
                            nc.scalar.activation(
                                e[:], stp[:], AF.Exp,
                                bias=zeros_col[:112], scale=SCALE,
                            )
                            nc.vector.tensor_mul(
                                em[:, 112 * hi : 112 * hi + 112], e[:],
                                mask2_t[:, 0:112])

                    def back():
                        wo = 112 * w
                        em = ems[0]
                        av = ppool.tile([65, 224], F32, tag="av", bufs=3,
                                        name=f"tav{s}_{w}_{hp}")
                        for hi in range(2):
                            h = 2 * hp + hi
                            nc.tensor.matmul(
                                av[:, 112 * hi : 112 * hi + 112],
                                st["vt"][w][:, 65 * h : 65 * h + 65],
                                em[:, 112 * hi : 112 * hi + 112],
                                start=True, stop=True,
                                skip_group_check=(hi == 1),
                            )
                        avc = spool.tile([64, 224], cdt, tag="avc", bufs=6,
                                         name=f"avct{s}_{w}_{hp}")
                        flex_copy(avc[:], av[0:64, :])
                        r = spool.tile([65, 224], cdt, tag="r", bufs=6,
                                       name=f"rt{s}_{w}_{hp}")
                        with nc.allow_low_precision(reason="1/softmax-sum"):
                            nc.vector.reciprocal(r[64:65, :], av[64:65, :])
                        bc = ppool.tile([64, 224], F32, tag="st", bufs=3,
                                        name=f"tbc{s}_{w}_{hp}")
                        nc.tensor.matmul(
                            bc[:], ones64[64:65, 0:64], r[64:65, :],
                            start=True, stop=True,
                        )
                        rb = spool.tile([64, 224], cdt, tag="rb", bufs=6,
                                        name=f"rbt{s}_{w}_{hp}")
                        flex_copy(rb[:], bc[:])
                        at = st["attnT"][3 + hp]
                        nc.vector.tensor_mul(
                            at[0:64, wo : wo + 112], avc[:, 0:112],
                            rb[:, 0:112],
                        )
                        tmp = spool.tile([64, 112], cdt, tag="tmp", bufs=6,
                                         name=f"tmt{s}_{w}_{hp}")
                        nc.vector.tensor_mul(tmp[:], avc[:, 112:224],
                                             rb[:, 112:224])
                        nc.sync.dma_start(at[64:128, wo : wo + 112], tmp[:])

                    return front, back

                for f in range(FPSB):
                    for hp in range(3):
                        groups.extend(spatial(f, hp))
                for w in range(WPSB):
                    for hp in range(3):
                        groups.extend(temporal(w, hp))
                return groups

            def o_chunks(s):
                """Output projection stream for superblock s."""
                so = SB * s
                st = sbt[s]
                chunks = []

                def o_chunk(ec, j):
                    def run():
                        ps = ppool.tile([128, 392], F32, tag="mm", bufs=2,
                                        name=f"ps_o{s}_{ec}_{j}")
                        for dc in range(6):
                            nc.tensor.matmul(
                                ps[:],
                                mmcast(wp[dc][:, 128 * ec : 128 * (ec + 1)]),
                                mmcast(st["attnT"][dc][:, 392 * j : 392 * (j + 1)]),
                                start=(dc == 0),
                                stop=(dc == 5),
                            )
                        ot = spool.tile([128, 392], F32, tag="ot", bufs=4,
                                        name=f"ot{s}_{ec}_{j}")
                        nc.scalar.activation(
                            ot[:], ps[:], AF.Identity,
                            bias=bias_t[:, ec : ec + 1], scale=1.0,
                        )
                        nc.sync.dma_start(
                            out_d.ap()[
                                128 * ec : 128 * (ec + 1),
                                so + 392 * j : so + 392 * (j + 1),
                            ],
                            ot[:],
                        )
                    return run

                for ec in range(6):
                    for j in range(2):
                        chunks.append(o_chunk(ec, j))
                return chunks

            def zip_emit(a_list, b_list):
                """Round-robin proportional interleave of two chunk lists."""
                na, nb = len(a_list), len(b_list)
                ia = ib = 0
                while ia < na or ib < nb:
                    if ib >= nb or (ia < na and ia * (nb + 1) <= ib * (na + 1)):
                        a_list[ia]()
                        ia += 1
                    else:
                        b_list[ib]()
                        ib += 1

            # software pipeline, rotated so every attention phase has
            # projection work zipped in: the loop body projects SB (s+1)%4
            # during attention of SB s — at s=3 that's SB0 of the NEXT
            # iteration (the prologue outside the loop primes SB0 once; the
            # tile tag rotation makes the in-loop P(0) land on the same
            # SBUF addresses, so iteration k+1's A(0) reads iteration k's
            # tail projections).
            if PIPE:
                for c in p_chunks(0):
                    c()
            rep_ctx = tc.For_i(0, reps, 1) if reps > 1 else contextlib.nullcontext()
            with rep_ctx:
                if PIPE:
                    for s in range(NSB):
                        others = p_chunks((s + 1) % NSB)
                        if s - 1 >= 0:
                            others += o_chunks(s - 1)
                        zip_emit(a_groups(s), others)
                    for c in o_chunks(NSB - 1):
                        c()
                else:
                    for s in range(NSB):
                        for c in p_chunks(s):
                            c()
                        for c in a_groups(s):
                            c()
                        for c in o_chunks(s):
                            c()

    nc.compile()
    return nc


def _get_nc(compute: str):
    if compute not in _CACHE:
        _CACHE[compute] = _build(compute)
    return _CACHE[compute]


def _np_dtype(compute: str):
    if compute == "f32":
        return np.float32
    import ml_dtypes

    return ml_dtypes.bfloat16


def _prep_in_maps(x, w_qkv, w_proj, b_proj, compute=None):
    dt = _np_dtype(compute or COMPUTE)
    x = np.asarray(x, dtype=np.float32).reshape(B, N, D)
    xT = np.ascontiguousarray(x.transpose(0, 2, 1)).astype(dt)  # (B, D, N)
    wqkvT = np.ascontiguousarray(np.asarray(w_qkv, np.float32).T).astype(dt)
    wprojT = np.ascontiguousarray(np.asarray(w_proj, np.float32).T).astype(dt)
    bias = np.asarray(b_proj, np.float32).reshape(D, 1)

    mask = np.zeros((112, 112), np.float32)
    for g in range(7):
        mask[16 * g : 16 * (g + 1), 16 * g : 16 * (g + 1)] = 1.0
    mask = mask.astype(dt)

    return [
        {"xt": xT[b], "wqkvT": wqkvT, "wprojT": wprojT, "bias": bias, "mask": mask}
        for b in range(B)
    ]


def _postprocess(results):
    out = np.stack([r["outT"].T for r in results])  # (B, N, D)
    return np.ascontiguousarray(out.reshape(B, F, P, D)).astype(np.float32)


def kernel(x, w_qkv, w_proj, b_proj):
    nc = _get_nc(COMPUTE)
    in_maps = _prep_in_maps(x, w_qkv, w_proj, b_proj)
    res = run_bass_kernel_spmd(nc, in_maps, core_ids=list(range(B)))
    return _postprocess(res.results)


if __name__ == "__main__":
    rng = np.random.default_rng(0)
    x = rng.standard_normal((B, F, P, D), dtype=np.float32)
    w_qkv = rng.standard_normal((E3, D), dtype=np.float32) * D**-0.5
    w_proj = rng.standard_normal((D, D), dtype=np.float32) * D**-0.5
    b_proj = np.zeros(D, np.float32)
    out = kernel(x=x, w_qkv=w_qkv, w_proj=w_proj, b_proj=b_proj)
    print(out.shape, out.dtype)


# revision 27
# speedup vs baseline: 1.1310x; 1.1310x over previous
"""Trainium2 Bass kernel for factorized space-time attention.

Computation (per batch b of 8, one NeuronCore each):
  qkv = x @ w_qkv.T                      (3136, 2304)
  heads 0-5:  spatial attention over 196 patches within each of 16 frames
  heads 6-11: temporal attention over groups of 16 consecutive tokens
              (raw-reshape semantics of the reference)
  out = concat(head outputs) @ w_proj.T + b_proj

Strategy: data-parallel over batch (8 cores). All activations kept
feature-major ([d, n]) on chip so every matmul contraction runs over the
partition dim with no on-device transposes; x / weights are pre-transposed
host-side. V is produced token-major directly by flipping the projection
matmul orientation (per-frame [128+68] row chunks for spatial heads,
112-row windows for temporal heads), with a ones column appended so the
AV matmul emits the softmax denominator for free. Temporal attention runs
on 112x112 score windows with a block-diagonal mask (7 x 16x16). Softmax
skips max-subtraction (scores ~N(0,1); exp is safe in fp32) and
normalizes via a ones-matmul partition-broadcast of 1/rowsum.

All matmul operands are bf16: TRN2 fp32 matmuls stream at 4 cycles/row vs
1 for bf16, so bf16 is ~4x faster on the PE; accumulation stays fp32 in
PSUM and the rel-err vs the fp32 reference is ~6e-3. Superblocks (784
tokens = lcm(196,16)) are double-buffered so projection DMA/copies of
SB s+1 overlap attention of SB s.
"""

import sys

if "/opt/trn_rl_repo" not in sys.path:
    sys.path.append("/opt/trn_rl_repo")

import numpy as np

import concourse.bass as bass  # noqa: F401
import concourse.mybir as mybir
import concourse.tile as tile
from concourse import bacc
from concourse.bass_utils import run_bass_kernel_spmd

F32 = mybir.dt.float32
BF16 = mybir.dt.bfloat16
AF = mybir.ActivationFunctionType

B = 8
F = 16
P = 196
D = 768
NH = 12
HD = 64
N = F * P
E3 = 3 * D
SB = 784
NSB = N // SB
FPSB = SB // P
WPSB = SB // 112
SCALE = HD ** -0.5

COMPUTE = "bf16"

_CACHE = {}


def _build(compute: str, reps: int = 1):
    cdt = BF16 if compute == "bf16" else F32
    F32R = mybir.dt.float32r

    def mmcast(ap):
        return ap.bitcast(F32R) if compute == "f32r" else ap

    wb = 2 if compute == "bf16" else 1

    nc = bacc.Bacc("TRN2", target_bir_lowering=False, debug=False, num_devices=B)

    xt_d = nc.dram_tensor("xt", (D, N), cdt, kind="ExternalInput")
    wqkv_d = nc.dram_tensor("wqkvT", (D, E3), cdt, kind="ExternalInput")
    wproj_d = nc.dram_tensor("wprojT", (D, D), cdt, kind="ExternalInput")
    bias_d = nc.dram_tensor("bias", (D, 1), F32, kind="ExternalInput")
    mask_d = nc.dram_tensor("mask", (112, 112), cdt, kind="ExternalInput")
    out_d = nc.dram_tensor("outT", (D, N), F32, kind="ExternalOutput")

    with tile.TileContext(nc) as tc:
        with (
            tc.tile_pool(name="const", bufs=1) as cpool,
            tc.tile_pool(name="work", bufs=1) as wpool,
            tc.tile_pool(name="small", bufs=4) as spool,
            tc.tile_pool(name="psum", bufs=2, space="PSUM") as ppool,
        ):
            wq = []
            for dc in range(6):
                t = cpool.tile([128, E3], cdt, tag=f"wq{dc}", name=f"wq{dc}")
                nc.sync.dma_start(t[:], wqkv_d.ap()[128 * dc : 128 * (dc + 1), :])
                wq.append(t)
            wp = []
            for dc in range(6):
                t = cpool.tile([128, D], cdt, tag=f"wp{dc}", name=f"wp{dc}")
                nc.sync.dma_start(t[:], wproj_d.ap()[128 * dc : 128 * (dc + 1), :])
                wp.append(t)
            bias_t = cpool.tile([128, 6], F32, tag="bias", name="bias_t")
            nc.sync.dma_start(
                bias_t[:], bias_d.ap().rearrange("(e p) one -> p (e one)", p=128)
            )
            mask2_t = cpool.tile([112, 224], cdt, tag="mask", name="mask2_t")
            nc.sync.dma_start(mask2_t[:, 0:112], mask_d.ap())
            nc.sync.dma_start(mask2_t[:, 112:224], mask_d.ap())
            zeros_col = cpool.tile([128, 1], F32, tag="zeros_c", name="zeros_col")
            nc.gpsimd.memset(zeros_col[:], 0.0)
            ones64 = cpool.tile([65, 64], cdt, tag="ones64", name="ones64")
            nc.gpsimd.memset(ones64[:], 1.0)

            import contextlib

            rep_ctx = tc.For_i(0, reps, 1) if reps > 1 else contextlib.nullcontext()
            with rep_ctx:
              for s in range(NSB):
                so = SB * s

                xts = []
                for dc in range(6):
                    t = wpool.tile([128, SB], cdt, tag=f"xts{dc}", bufs=wb, name=f"xts{dc}_{s}")
                    nc.sync.dma_start(
                        t[:], xt_d.ap()[128 * dc : 128 * (dc + 1), so : so + SB]
                    )
                    xts.append(t)

                qkvt = []
                for ti in range(12):
                    qt = wpool.tile([128, SB], cdt, tag=f"qkvt{ti}", bufs=wb, name=f"qkvt{ti}_{s}")
                    for j in range(2):
                        ps = ppool.tile([128, 392], F32, tag="mm", bufs=2, name=f"ps_qk{s}_{ti}_{j}")
                        for dc in range(6):
                            nc.tensor.matmul(
                                ps[:],
                                mmcast(wq[dc][:, 128 * ti : 128 * (ti + 1)]),
                                mmcast(xts[dc][:, 392 * j : 392 * (j + 1)]),
                                start=(dc == 0),
                                stop=(dc == 5),
                            )
                        nc.scalar.copy(qt[:, 392 * j : 392 * (j + 1)], ps[:])
                    qkvt.append(qt)

                def v_proj(msz, tok0, wcol0, vtag, vname, psname):
                    vt_ = wpool.tile([msz, 390], cdt, tag=vtag, bufs=wb, name=vname)
                    ps = ppool.tile([msz, 384], F32, tag="mm", bufs=2, name=psname)
                    for dc in range(6):
                        nc.tensor.matmul(
                            ps[:],
                            mmcast(xts[dc][:, tok0 : tok0 + msz]),
                            mmcast(wq[dc][:, wcol0 : wcol0 + 384]),
                            start=(dc == 0),
                            stop=(dc == 5),
                        )
                    nc.scalar.copy(
                        vt_.rearrange("p (h c) -> p h c", c=65)[:, :, 0:64],
                        ps.rearrange("p (h c) -> p h c", c=64),
                    )
                    nc.gpsimd.memset(
                        vt_.rearrange("p (h c) -> p h c", c=65)[:, :, 64:65], 1.0
                    )
                    return vt_

                vs = []
                for f in range(FPSB):
                    for ci, (m0, msz) in enumerate(((0, 128), (128, 68))):
                        vs.append(
                            v_proj(msz, 196 * f + m0, 1536, f"vs{f}_{ci}",
                                   f"vs{f}_{ci}_{s}", f"ps_vs{s}_{f}_{ci}")
                        )
                vt = []
                for w in range(WPSB):
                    vt.append(
                        v_proj(112, 112 * w, 1920, f"vt{w}",
                               f"vt{w}_{s}", f"ps_vt{s}_{w}")
                    )

                attnT = [
                    wpool.tile([128, SB], cdt, tag=f"attnT{i}", bufs=wb,
                               name=f"attnT{i}_{s}")
                    for i in range(6)
                ]

                for f in range(FPSB):
                    fo = 196 * f
                    for hp in range(3):
                        ps_avs = []
                        for hi in range(2):
                            h = 2 * hp + hi
                            pb = 64 * hi
                            qtile = qkvt[h // 2]
                            ktile = qkvt[6 + h // 2]
                            es = []
                            for ci, (m0, msz) in enumerate(((0, 128), (128, 68))):
                                ps_st = ppool.tile(
                                    [msz, 196], F32, tag="st", bufs=3,
                                    name=f"ps_st{s}_{f}_{h}_{ci}",
                                )
                                nc.tensor.matmul(
                                    ps_st[:],
                                    ktile[pb : pb + 64, fo + m0 : fo + m0 + msz],
                                    qtile[pb : pb + 64, fo : fo + 196],
                                    start=True,
                                    stop=True,
                                )
                                e = spool.tile(
                                    [msz, 196], cdt, tag="e", bufs=6,
                                    name=f"e{s}_{f}_{h}_{ci}",
                                )
                                nc.scalar.activation(
                                    e[:], ps_st[:], AF.Exp,
                                    bias=zeros_col[:msz, :], scale=SCALE,
                                )
                                es.append(e)
                            ps_av = ppool.tile(
                                [65, 196], F32, tag="av", bufs=2,
                                name=f"ps_sav{s}_{f}_{h}",
                            )
                            for ci in range(2):
                                nc.tensor.matmul(
                                    ps_av[:],
                                    vs[2 * f + ci][:, 65 * h : 65 * h + 65],
                                    es[ci][:],
                                    start=(ci == 0),
                                    stop=(ci == 1),
                                )
                            ps_avs.append(ps_av)
                        r = spool.tile([65, 392], cdt, tag="r", name=f"r{s}_{f}_{hp}")
                        with nc.allow_low_precision(reason="1/softmax-sum in cdt"):
                            for hi in range(2):
                                nc.vector.reciprocal(
                                    r[64:65, 196 * hi : 196 * hi + 196],
                                    ps_avs[hi][64:65, :],
                                )
                        ps_b = ppool.tile(
                            [64, 392], F32, tag="mm", bufs=2, name=f"ps_b{s}_{f}_{hp}"
                        )
                        nc.tensor.matmul(
                            ps_b[:], ones64[64:65, :], r[64:65, :],
                            start=True, stop=True,
                        )
                        rb = spool.tile([64, 392], F32, tag="rb", name=f"rb{s}_{f}_{hp}")
                        nc.scalar.copy(rb[:], ps_b[:])
                        for hi in range(2):
                            h = 2 * hp + hi
                            cs = slice(196 * hi, 196 * hi + 196)
                            if hi == 0:
                                nc.vector.tensor_mul(
                                    attnT[h // 2][0:64, fo : fo + 196],
                                    ps_avs[hi][0:64, :], rb[:, cs],
                                )
                            else:
                                tmp = spool.tile(
                                    [64, 196], cdt, tag="tmp", name=f"tm{s}_{f}_{h}"
                                )
                                nc.vector.tensor_mul(
                                    tmp[:], ps_avs[hi][0:64, :], rb[:, cs]
                                )
                                nc.sync.dma_start(
                                    attnT[h // 2][64:128, fo : fo + 196], tmp[:]
                                )

                for w in range(WPSB):
                    wo = 112 * w
                    for hp in range(3):
                        ps_avs = []
                        for hi in range(2):
                            h = 6 + 2 * hp + hi
                            pb = 64 * hi
                            ps_st = ppool.tile(
                                [112, 112], F32, tag="st", bufs=3,
                                name=f"ps_tst{s}_{w}_{h}",
                            )
                            nc.tensor.matmul(
                                ps_st[:],
                                qkvt[6 + h // 2][pb : pb + 64, wo : wo + 112],
                                qkvt[h // 2][pb : pb + 64, wo : wo + 112],
                                start=True,
                                stop=True,
                            )
                            e = spool.tile(
                                [112, 112], cdt, tag="e", bufs=6,
                                name=f"et{s}_{w}_{h}",
                            )
                            nc.scalar.activation(
                                e[:], ps_st[:], AF.Exp,
                                bias=zeros_col[:112], scale=SCALE,
                            )
                            em = spool.tile(
                                [112, 112], cdt, tag="e", bufs=6,
                                name=f"em{s}_{w}_{h}",
                            )
                            nc.vector.tensor_mul(em[:], e[:], mask2_t[:, 0:112])
                            ps_av = ppool.tile(
                                [65, 112], F32, tag="av", bufs=2,
                                name=f"ps_tav{s}_{w}_{h}",
                            )
                            nc.tensor.matmul(
                                ps_av[:],
                                vt[w][:, 65 * (h - 6) : 65 * (h - 6) + 65],
                                em[:],
                                start=True,
                                stop=True,
                            )
                            ps_avs.append(ps_av)
                        r = spool.tile([65, 224], cdt, tag="r", name=f"rt{s}_{w}_{hp}")
                        with nc.allow_low_precision(reason="1/softmax-sum in cdt"):
                            for hi in range(2):
                                nc.vector.reciprocal(
                                    r[64:65, 112 * hi : 112 * hi + 112],
                                    ps_avs[hi][64:65, :],
                                )
                        ps_b = ppool.tile(
                            [64, 224], F32, tag="mm", bufs=2, name=f"ps_tb{s}_{w}_{hp}"
                        )
                        nc.tensor.matmul(
                            ps_b[:], ones64[64:65, :], r[64:65, :],
                            start=True, stop=True,
                        )
                        rb = spool.tile([64, 224], F32, tag="rb", name=f"rbt{s}_{w}_{hp}")
                        nc.scalar.copy(rb[:], ps_b[:])
                        for hi in range(2):
                            h = 6 + 2 * hp + hi
                            cs = slice(112 * hi, 112 * hi + 112)
                            at = attnT[3 + (h - 6) // 2]
                            if hi == 0:
                                nc.vector.tensor_mul(
                                    at[0:64, wo : wo + 112], ps_avs[hi][0:64, :],
                                    rb[:, cs],
                                )
                            else:
                                tmp = spool.tile(
                                    [64, 112], cdt, tag="tmp", name=f"tmt{s}_{w}_{h}"
                                )
                                nc.vector.tensor_mul(
                                    tmp[:], ps_avs[hi][0:64, :], rb[:, cs]
                                )
                                nc.sync.dma_start(
                                    at[64:128, wo : wo + 112], tmp[:]
                                )

                for ec in range(6):
                    for j in range(2):
                        ps = ppool.tile([128, 392], F32, tag="mm", bufs=2, name=f"ps_o{s}_{ec}_{j}")
                        for dc in range(6):
                            nc.tensor.matmul(
                                ps[:],
                                mmcast(wp[dc][:, 128 * ec : 128 * (ec + 1)]),
                                mmcast(attnT[dc][:, 392 * j : 392 * (j + 1)]),
                                start=(dc == 0),
                                stop=(dc == 5),
                            )
                        ot = spool.tile([128, 392], F32, tag="ot", name=f"ot{s}_{ec}_{j}")
                        nc.scalar.activation(
                            ot[:], ps[:], AF.Identity,
                            bias=bias_t[:, ec : ec + 1], scale=1.0,
                        )
                        nc.sync.dma_start(
                            out_d.ap()[
                                128 * ec : 128 * (ec + 1),
                                so + 392 * j : so + 392 * (j + 1),
                            ],
                            ot[:],
                        )

    nc.compile()
    return nc


def _get_nc(compute: str):
    if compute not in _CACHE:
        _CACHE[compute] = _build(compute)
    return _CACHE[compute]


def _np_dtype(compute: str):
    if compute == "f32":
        return np.float32
    import ml_dtypes

    return ml_dtypes.bfloat16


def _prep_in_maps(x, w_qkv, w_proj, b_proj, compute=None):
    dt = _np_dtype(compute or COMPUTE)
    x = np.asarray(x, dtype=np.float32).reshape(B, N, D)
    xT = np.ascontiguousarray(x.transpose(0, 2, 1)).astype(dt)
    wqkvT = np.ascontiguousarray(np.asarray(w_qkv, np.float32).T).astype(dt)
    wprojT = np.ascontiguousarray(np.asarray(w_proj, np.float32).T).astype(dt)
    bias = np.asarray(b_proj, np.float32).reshape(D, 1)

    mask = np.zeros((112, 112), np.float32)
    for g in range(7):
        mask[16 * g : 16 * (g + 1), 16 * g : 16 * (g + 1)] = 1.0
    mask = mask.astype(dt)

    return [
        {"xt": xT[b], "wqkvT": wqkvT, "wprojT": wprojT, "bias": bias, "mask": mask}
        for b in range(B)
    ]


def _postprocess(results):
    out = np.stack([r["outT"].T for r in results])
    return np.ascontiguousarray(out.reshape(B, F, P, D)).astype(np.float32)


def kernel(x, w_qkv, w_proj, b_proj):
    nc = _get_nc(COMPUTE)
    in_maps = _prep_in_maps(x, w_qkv, w_proj, b_proj)
    res = run_bass_kernel_spmd(nc, in_maps, core_ids=list(range(B)))
    return _postprocess(res.results)


# revision 31
# speedup vs baseline: 1.1408x; 1.0087x over previous
"""Trainium2 Bass kernel for factorized space-time attention.

Computation (per batch b of 8, one NeuronCore each):
  qkv = x @ w_qkv.T                      (3136, 2304)
  heads 0-5:  spatial attention over 196 patches within each of 16 frames
  heads 6-11: temporal attention over groups of 16 consecutive tokens
              (raw-reshape semantics of the reference)
  out = concat(head outputs) @ w_proj.T + b_proj

Strategy: data-parallel over batch (8 cores). All activations kept
feature-major ([d, n]) on chip so every matmul contraction runs over the
partition dim with no on-device transposes; x / weights are pre-transposed
host-side. V is produced token-major directly by flipping the projection
matmul orientation (per-frame [128+68] row chunks for spatial heads,
112-row windows for temporal heads), with a ones column appended so the
AV matmul emits the softmax denominator for free. Temporal attention runs
on 112x112 score windows with a block-diagonal mask (7 x 16x16). Softmax
skips max-subtraction (scores ~N(0,1); exp is safe in fp32) and
normalizes via a ones-matmul partition-broadcast of 1/rowsum.

All matmul operands are bf16: TRN2 fp32 matmuls stream at 4 cycles/row vs
1 for bf16, so bf16 is ~4x faster on the PE; accumulation stays fp32 in
PSUM and the rel-err vs the fp32 reference is ~6e-3. Superblocks (784
tokens = lcm(196,16)) are double-buffered so projection DMA/copies of
SB s+1 overlap attention of SB s.
"""

import sys

if "/opt/trn_rl_repo" not in sys.path:
    sys.path.append("/opt/trn_rl_repo")

import numpy as np

import concourse.bass as bass  # noqa: F401
import concourse.mybir as mybir
import concourse.tile as tile
from concourse import bacc
from concourse.bass_utils import run_bass_kernel_spmd

F32 = mybir.dt.float32
BF16 = mybir.dt.bfloat16
AF = mybir.ActivationFunctionType

B = 8
F = 16
P = 196
D = 768
NH = 12
HD = 64
N = F * P
E3 = 3 * D
SB = 784
NSB = N // SB
FPSB = SB // P
WPSB = SB // 112
SCALE = HD ** -0.5

COMPUTE = "bf16"

_CACHE = {}


def _build(compute: str, reps: int = 1):
    cdt = BF16 if compute == "bf16" else F32
    F32R = mybir.dt.float32r

    def mmcast(ap):
        return ap.bitcast(F32R) if compute == "f32r" else ap

    wb = 2 if compute == "bf16" else 1

    nc = bacc.Bacc("TRN2", target_bir_lowering=False, debug=False, num_devices=B)

    xt_d = nc.dram_tensor("xt", (D, N), cdt, kind="ExternalInput")
    wqkv_d = nc.dram_tensor("wqkvT", (D, E3), cdt, kind="ExternalInput")
    wproj_d = nc.dram_tensor("wprojT", (D, D), cdt, kind="ExternalInput")
    bias_d = nc.dram_tensor("bias", (D, 1), F32, kind="ExternalInput")
    mask_d = nc.dram_tensor("mask", (112, 112), cdt, kind="ExternalInput")
    out_d = nc.dram_tensor("outT", (D, N), F32, kind="ExternalOutput")

    with tile.TileContext(nc) as tc:
        with (
            tc.tile_pool(name="const", bufs=1) as cpool,
            tc.tile_pool(name="work", bufs=1) as wpool,
            tc.tile_pool(name="small", bufs=4) as spool,
            tc.tile_pool(name="psum", bufs=2, space="PSUM") as ppool,
        ):
            wq = []
            for dc in range(6):
                t = cpool.tile([128, E3], cdt, tag=f"wq{dc}", name=f"wq{dc}")
                nc.sync.dma_start(t[:], wqkv_d.ap()[128 * dc : 128 * (dc + 1), :])
                wq.append(t)
            wp = []
            for dc in range(6):
                t = cpool.tile([128, D], cdt, tag=f"wp{dc}", name=f"wp{dc}")
                nc.sync.dma_start(t[:], wproj_d.ap()[128 * dc : 128 * (dc + 1), :])
                wp.append(t)
            bias_t = cpool.tile([128, 6], F32, tag="bias", name="bias_t")
            nc.sync.dma_start(
                bias_t[:], bias_d.ap().rearrange("(e p) one -> p (e one)", p=128)
            )
            mask2_t = cpool.tile([112, 224], cdt, tag="mask", name="mask2_t")
            nc.sync.dma_start(mask2_t[:, 0:112], mask_d.ap())
            nc.sync.dma_start(mask2_t[:, 112:224], mask_d.ap())
            zeros_col = cpool.tile([128, 1], F32, tag="zeros_c", name="zeros_col")
            nc.gpsimd.memset(zeros_col[:], 0.0)
            ones64 = cpool.tile([65, 64], cdt, tag="ones64", name="ones64")
            nc.gpsimd.memset(ones64[:], 1.0)

            flex_ctr = [0]

            def flex_copy(dst, src):
                flex_ctr[0] += 1
                if flex_ctr[0] % 2 == 0:
                    nc.scalar.copy(dst, src)
                else:
                    nc.vector.tensor_copy(dst, src)

            import contextlib

            rep_ctx = tc.For_i(0, reps, 1) if reps > 1 else contextlib.nullcontext()
            with rep_ctx:
              for s in range(NSB):
                so = SB * s

                xts = []
                for dc in range(6):
                    t = wpool.tile([128, SB], cdt, tag=f"xts{dc}", bufs=wb, name=f"xts{dc}_{s}")
                    nc.sync.dma_start(
                        t[:], xt_d.ap()[128 * dc : 128 * (dc + 1), so : so + SB]
                    )
                    xts.append(t)

                qkvt = []
                for ti in range(12):
                    qt = wpool.tile([128, SB], cdt, tag=f"qkvt{ti}", bufs=wb, name=f"qkvt{ti}_{s}")
                    for j in range(2):
                        ps = ppool.tile([128, 392], F32, tag="mm", bufs=2, name=f"ps_qk{s}_{ti}_{j}")
                        for dc in range(6):
                            nc.tensor.matmul(
                                ps[:],
                                mmcast(wq[dc][:, 128 * ti : 128 * (ti + 1)]),
                                mmcast(xts[dc][:, 392 * j : 392 * (j + 1)]),
                                start=(dc == 0),
                                stop=(dc == 5),
                            )
                        flex_copy(qt[:, 392 * j : 392 * (j + 1)], ps[:])
                    qkvt.append(qt)

                def v_proj(msz, tok0, wcol0, vtag, vname, psname):
                    vt_ = wpool.tile([msz, 390], cdt, tag=vtag, bufs=wb, name=vname)
                    ps = ppool.tile([msz, 384], F32, tag="mm", bufs=2, name=psname)
                    for dc in range(6):
                        nc.tensor.matmul(
                            ps[:],
                            mmcast(xts[dc][:, tok0 : tok0 + msz]),
                            mmcast(wq[dc][:, wcol0 : wcol0 + 384]),
                            start=(dc == 0),
                            stop=(dc == 5),
                        )
                    flex_copy(
                        vt_.rearrange("p (h c) -> p h c", c=65)[:, :, 0:64],
                        ps.rearrange("p (h c) -> p h c", c=64),
                    )
                    nc.gpsimd.memset(
                        vt_.rearrange("p (h c) -> p h c", c=65)[:, :, 64:65], 1.0
                    )
                    return vt_

                vs = []
                for f in range(FPSB):
                    for ci, (m0, msz) in enumerate(((0, 128), (128, 68))):
                        vs.append(
                            v_proj(msz, 196 * f + m0, 1536, f"vs{f}_{ci}",
                                   f"vs{f}_{ci}_{s}", f"ps_vs{s}_{f}_{ci}")
                        )
                vt = []
                for w in range(WPSB):
                    vt.append(
                        v_proj(112, 112 * w, 1920, f"vt{w}",
                               f"vt{w}_{s}", f"ps_vt{s}_{w}")
                    )

                attnT = [
                    wpool.tile([128, SB], cdt, tag=f"attnT{i}", bufs=wb,
                               name=f"attnT{i}_{s}")
                    for i in range(6)
                ]

                for f in range(FPSB):
                    fo = 196 * f
                    for hp in range(3):
                        ps_avs = []
                        for hi in range(2):
                            h = 2 * hp + hi
                            pb = 64 * hi
                            qtile = qkvt[h // 2]
                            ktile = qkvt[6 + h // 2]
                            es = []
                            for ci, (m0, msz) in enumerate(((0, 128), (128, 68))):
                                ps_st = ppool.tile(
                                    [msz, 196], F32, tag="st", bufs=3,
                                    name=f"ps_st{s}_{f}_{h}_{ci}",
                                )
                                nc.tensor.matmul(
                                    ps_st[:],
                                    ktile[pb : pb + 64, fo + m0 : fo + m0 + msz],
                                    qtile[pb : pb + 64, fo : fo + 196],
                                    start=True,
                                    stop=True,
                                )
                                e = spool.tile(
                                    [msz, 196], cdt, tag="e", bufs=6,
                                    name=f"e{s}_{f}_{h}_{ci}",
                                )
                                nc.scalar.activation(
                                    e[:], ps_st[:], AF.Exp,
                                    bias=zeros_col[:msz, :], scale=SCALE,
                                )
                                es.append(e)
                            ps_av = ppool.tile(
                                [65, 196], F32, tag="av", bufs=2,
                                name=f"ps_sav{s}_{f}_{h}",
                            )
                            for ci in range(2):
                                nc.tensor.matmul(
                                    ps_av[:],
                                    vs[2 * f + ci][:, 65 * h : 65 * h + 65],
                                    es[ci][:],
                                    start=(ci == 0),
                                    stop=(ci == 1),
                                )
                            ps_avs.append(ps_av)
                        r = spool.tile([65, 392], cdt, tag="r", name=f"r{s}_{f}_{hp}")
                        with nc.allow_low_precision(reason="1/softmax-sum in cdt"):
                            for hi in range(2):
                                nc.vector.reciprocal(
                                    r[64:65, 196 * hi : 196 * hi + 196],
                                    ps_avs[hi][64:65, :],
                                )
                        ps_b = ppool.tile(
                            [64, 392], F32, tag="mm", bufs=2, name=f"ps_b{s}_{f}_{hp}"
                        )
                        nc.tensor.matmul(
                            ps_b[:], ones64[64:65, :], r[64:65, :],
                            start=True, stop=True,
                        )
                        rb = spool.tile([64, 392], cdt, tag="rb", name=f"rb{s}_{f}_{hp}")
                        flex_copy(rb[:], ps_b[:])
                        for hi in range(2):
                            h = 2 * hp + hi
                            cs = slice(196 * hi, 196 * hi + 196)
                            avc = spool.tile([64, 196], cdt, tag="avc", bufs=6,
                                             name=f"avc{s}_{f}_{h}")
                            flex_copy(avc[:], ps_avs[hi][0:64, :])
                            if hi == 0:
                                nc.vector.tensor_mul(
                                    attnT[h // 2][0:64, fo : fo + 196],
                                    avc[:], rb[:, cs],
                                )
                            else:
                                tmp = spool.tile(
                                    [64, 196], cdt, tag="tmp", name=f"tm{s}_{f}_{h}"
                                )
                                nc.vector.tensor_mul(
                                    tmp[:], avc[:], rb[:, cs]
                                )
                                nc.sync.dma_start(
                                    attnT[h // 2][64:128, fo : fo + 196], tmp[:]
                                )

                for w in range(WPSB):
                    wo = 112 * w
                    for hp in range(3):
                        ps_avs = []
                        for hi in range(2):
                            h = 6 + 2 * hp + hi
                            pb = 64 * hi
                            ps_st = ppool.tile(
                                [112, 112], F32, tag="st", bufs=3,
                                name=f"ps_tst{s}_{w}_{h}",
                            )
                            nc.tensor.matmul(
                                ps_st[:],
                                qkvt[6 + h // 2][pb : pb + 64, wo : wo + 112],
                                qkvt[h // 2][pb : pb + 64, wo : wo + 112],
                                start=True,
                                stop=True,
                            )
                            e = spool.tile(
                                [112, 112], cdt, tag="e", bufs=6,
                                name=f"et{s}_{w}_{h}",
                            )
                            nc.scalar.activation(
                                e[:], ps_st[:], AF.Exp,
                                bias=zeros_col[:112], scale=SCALE,
                            )
                            em = spool.tile(
                                [112, 112], cdt, tag="e", bufs=6,
                                name=f"em{s}_{w}_{h}",
                            )
                            nc.vector.tensor_mul(em[:], e[:], mask2_t[:, 0:112])
                            ps_av = ppool.tile(
                                [65, 112], F32, tag="av", bufs=2,
                                name=f"ps_tav{s}_{w}_{h}",
                            )
                            nc.tensor.matmul(
                                ps_av[:],
                                vt[w][:, 65 * (h - 6) : 65 * (h - 6) + 65],
                                em[:],
                                start=True,
                                stop=True,
                            )
                            ps_avs.append(ps_av)
                        r = spool.tile([65, 224], cdt, tag="r", name=f"rt{s}_{w}_{hp}")
                        with nc.allow_low_precision(reason="1/softmax-sum in cdt"):
                            for hi in range(2):
                                nc.vector.reciprocal(
                                    r[64:65, 112 * hi : 112 * hi + 112],
                                    ps_avs[hi][64:65, :],
                                )
                        ps_b = ppool.tile(
                            [64, 224], F32, tag="mm", bufs=2, name=f"ps_tb{s}_{w}_{hp}"
                        )
                        nc.tensor.matmul(
                            ps_b[:], ones64[64:65, :], r[64:65, :],
                            start=True, stop=True,
                        )
                        rb = spool.tile([64, 224], cdt, tag="rb", name=f"rbt{s}_{w}_{hp}")
                        flex_copy(rb[:], ps_b[:])
                        for hi in range(2):
                            h = 6 + 2 * hp + hi
                            cs = slice(112 * hi, 112 * hi + 112)
                            at = attnT[3 + (h - 6) // 2]
                            avc = spool.tile([64, 112], cdt, tag="avc", bufs=6,
                                             name=f"avct{s}_{w}_{h}")
                            flex_copy(avc[:], ps_avs[hi][0:64, :])
                            if hi == 0:
                                nc.vector.tensor_mul(
                                    at[0:64, wo : wo + 112], avc[:],
                                    rb[:, cs],
                                )
                            else:
                                tmp = spool.tile(
                                    [64, 112], cdt, tag="tmp", name=f"tmt{s}_{w}_{h}"
                                )
                                nc.vector.tensor_mul(
                                    tmp[:], avc[:], rb[:, cs]
                                )
                                nc.sync.dma_start(
                                    at[64:128, wo : wo + 112], tmp[:]
                                )

                for ec in range(6):
                    for j in range(2):
                        ps = ppool.tile([128, 392], F32, tag="mm", bufs=2, name=f"ps_o{s}_{ec}_{j}")
                        for dc in range(6):
                            nc.tensor.matmul(
                                ps[:],
                                mmcast(wp[dc][:, 128 * ec : 128 * (ec + 1)]),
                                mmcast(attnT[dc][:, 392 * j : 392 * (j + 1)]),
                                start=(dc == 0),
                                stop=(dc == 5),
                            )
                        ot = spool.tile([128, 392], F32, tag="ot", name=f"ot{s}_{ec}_{j}")
                        nc.scalar.activation(
                            ot[:], ps[:], AF.Identity,
                            bias=bias_t[:, ec : ec + 1], scale=1.0,
                        )
                        nc.sync.dma_start(
                            out_d.ap()[
                                128 * ec : 128 * (ec + 1),
                                so + 392 * j : so + 392 * (j + 1),
                            ],
                            ot[:],
                        )

    nc.compile()
    return nc


def _get_nc(compute: str):
    if compute not in _CACHE:
        _CACHE[compute] = _build(compute)
    return _CACHE[compute]


def _np_dtype(compute: str):
    if compute == "f32":
        return np.float32
    import ml_dtypes

    return ml_dtypes.bfloat16


def _prep_in_maps(x, w_qkv, w_proj, b_proj, compute=None):
    dt = _np_dtype(compute or COMPUTE)
    x = np.asarray(x, dtype=np.float32).reshape(B, N, D)
    xT = np.ascontiguousarray(x.transpose(0, 2, 1)).astype(dt)
    wqkvT = np.ascontiguousarray(np.asarray(w_qkv, np.float32).T).astype(dt)
    wprojT = np.ascontiguousarray(np.asarray(w_proj, np.float32).T).astype(dt)
    bias = np.asarray(b_proj, np.float32).reshape(D, 1)

    mask = np.zeros((112, 112), np.float32)
    for g in range(7):
        mask[16 * g : 16 * (g + 1), 16 * g : 16 * (g + 1)] = 1.0
    mask = mask.astype(dt)

    return [
        {"xt": xT[b], "wqkvT": wqkvT, "wprojT": wprojT, "bias": bias, "mask": mask}
        for b in range(B)
    ]


def _postprocess(results):
    out = np.stack([r["outT"].T for r in results])
    return np.ascontiguousarray(out.reshape(B, F, P, D)).astype(np.float32)


def kernel(x, w_qkv, w_proj, b_proj):
    nc = _get_nc(COMPUTE)
    in_maps = _prep_in_maps(x, w_qkv, w_proj, b_proj)
    res = run_bass_kernel_spmd(nc, in_maps, core_ids=list(range(B)))
    return _postprocess(res.results)


# revision 32
# speedup vs baseline: 1.1813x; 1.0355x over previous
"""Trainium2 Bass kernel for factorized space-time attention.

Computation (per batch b of 8, one NeuronCore each):
  qkv = x @ w_qkv.T                      (3136, 2304)
  heads 0-5:  spatial attention over 196 patches within each of 16 frames
  heads 6-11: temporal attention over groups of 16 consecutive tokens
              (raw-reshape semantics of the reference)
  out = concat(head outputs) @ w_proj.T + b_proj

Strategy: data-parallel over batch (8 cores). All activations kept
feature-major ([d, n]) on chip so every matmul contraction runs over the
partition dim with no on-device transposes; x / weights are pre-transposed
host-side. V is produced token-major directly by flipping the projection
matmul orientation (per-frame [128+68] row chunks for spatial heads,
112-row windows for temporal heads), with a ones column appended so the
AV matmul emits the softmax denominator for free. Temporal attention runs
on 112x112 score windows with a block-diagonal mask (7 x 16x16). Softmax
skips max-subtraction (scores ~N(0,1); exp is safe in fp32) and
normalizes via a ones-matmul partition-broadcast of 1/rowsum.

All matmul operands are bf16: TRN2 fp32 matmuls stream at 4 cycles/row vs
1 for bf16, so bf16 is ~4x faster on the PE; accumulation stays fp32 in
PSUM and the rel-err vs the fp32 reference is ~6e-3. Superblocks (784
tokens = lcm(196,16)) are double-buffered so projection DMA/copies of
SB s+1 overlap attention of SB s.
"""

import sys

if "/opt/trn_rl_repo" not in sys.path:
    sys.path.append("/opt/trn_rl_repo")

import numpy as np

import concourse.bass as bass  # noqa: F401
import concourse.mybir as mybir
import concourse.tile as tile
from concourse import bacc
from concourse.bass_utils import run_bass_kernel_spmd

F32 = mybir.dt.float32
BF16 = mybir.dt.bfloat16
AF = mybir.ActivationFunctionType

B = 8
F = 16
P = 196
D = 768
NH = 12
HD = 64
N = F * P
E3 = 3 * D
SB = 784
NSB = N // SB
FPSB = SB // P
WPSB = SB // 112
SCALE = HD ** -0.5

COMPUTE = "bf16"

_CACHE = {}


def _build(compute: str, reps: int = 1):
    cdt = BF16 if compute == "bf16" else F32
    F32R = mybir.dt.float32r

    def mmcast(ap):
        return ap.bitcast(F32R) if compute == "f32r" else ap

    wb = 2 if compute == "bf16" else 1

    nc = bacc.Bacc("TRN2", target_bir_lowering=False, debug=False, num_devices=B)

    xt_d = nc.dram_tensor("xt", (D, N), cdt, kind="ExternalInput")
    wqkv_d = nc.dram_tensor("wqkvT", (D, E3), cdt, kind="ExternalInput")
    wproj_d = nc.dram_tensor("wprojT", (D, D), cdt, kind="ExternalInput")
    bias_d = nc.dram_tensor("bias", (D, 1), F32, kind="ExternalInput")
    mask_d = nc.dram_tensor("mask", (112, 112), cdt, kind="ExternalInput")
    out_d = nc.dram_tensor("outT", (D, N), F32, kind="ExternalOutput")

    with tile.TileContext(nc) as tc:
        with (
            tc.tile_pool(name="const", bufs=1) as cpool,
            tc.tile_pool(name="work", bufs=1) as wpool,
            tc.tile_pool(name="small", bufs=4) as spool,
            tc.tile_pool(name="psum", bufs=2, space="PSUM") as ppool,
        ):
            wq = []
            for dc in range(6):
                t = cpool.tile([128, E3], cdt, tag=f"wq{dc}", name=f"wq{dc}")
                nc.sync.dma_start(t[:], wqkv_d.ap()[128 * dc : 128 * (dc + 1), :])
                wq.append(t)
            wp = []
            for dc in range(6):
                t = cpool.tile([128, D], cdt, tag=f"wp{dc}", name=f"wp{dc}")
                nc.sync.dma_start(t[:], wproj_d.ap()[128 * dc : 128 * (dc + 1), :])
                wp.append(t)
            bias_t = cpool.tile([128, 6], F32, tag="bias", name="bias_t")
            nc.sync.dma_start(
                bias_t[:], bias_d.ap().rearrange("(e p) one -> p (e one)", p=128)
            )
            mask2_t = cpool.tile([112, 224], cdt, tag="mask", name="mask2_t")
            nc.sync.dma_start(mask2_t[:, 0:112], mask_d.ap())
            nc.sync.dma_start(mask2_t[:, 112:224], mask_d.ap())
            zeros_col = cpool.tile([128, 1], F32, tag="zeros_c", name="zeros_col")
            nc.gpsimd.memset(zeros_col[:], 0.0)
            ones64 = cpool.tile([65, 64], cdt, tag="ones64", name="ones64")
            nc.gpsimd.memset(ones64[:], 1.0)

            flex_ctr = [0]

            def flex_copy(dst, src):
                flex_ctr[0] += 1
                if flex_ctr[0] % 2 == 0:
                    nc.scalar.copy(dst, src)
                else:
                    nc.vector.tensor_copy(dst, src)

            import contextlib

            rep_ctx = tc.For_i(0, reps, 1) if reps > 1 else contextlib.nullcontext()
            with rep_ctx:
              for s in range(NSB):
                so = SB * s

                xts = []
                for dc in range(6):
                    t = wpool.tile([128, SB], cdt, tag=f"xts{dc}", bufs=wb, name=f"xts{dc}_{s}")
                    nc.sync.dma_start(
                        t[:], xt_d.ap()[128 * dc : 128 * (dc + 1), so : so + SB]
                    )
                    xts.append(t)

                qkvt = []
                for ti in range(12):
                    qt = wpool.tile([128, SB], cdt, tag=f"qkvt{ti}", bufs=wb, name=f"qkvt{ti}_{s}")
                    for j in range(2):
                        ps = ppool.tile([128, 392], F32, tag="mm", bufs=2, name=f"ps_qk{s}_{ti}_{j}")
                        for dc in range(6):
                            nc.tensor.matmul(
                                ps[:],
                                mmcast(wq[dc][:, 128 * ti : 128 * (ti + 1)]),
                                mmcast(xts[dc][:, 392 * j : 392 * (j + 1)]),
                                start=(dc == 0),
                                stop=(dc == 5),
                            )
                        flex_copy(qt[:, 392 * j : 392 * (j + 1)], ps[:])
                    qkvt.append(qt)

                def v_proj(msz, tok0, wcol0, vtag, vname, psname):
                    vt_ = wpool.tile([msz, 390], cdt, tag=vtag, bufs=wb, name=vname)
                    ps = ppool.tile([msz, 384], F32, tag="mm", bufs=2, name=psname)
                    for dc in range(6):
                        nc.tensor.matmul(
                            ps[:],
                            mmcast(xts[dc][:, tok0 : tok0 + msz]),
                            mmcast(wq[dc][:, wcol0 : wcol0 + 384]),
                            start=(dc == 0),
                            stop=(dc == 5),
                        )
                    flex_copy(
                        vt_.rearrange("p (h c) -> p h c", c=65)[:, :, 0:64],
                        ps.rearrange("p (h c) -> p h c", c=64),
                    )
                    nc.gpsimd.memset(
                        vt_.rearrange("p (h c) -> p h c", c=65)[:, :, 64:65], 1.0
                    )
                    return vt_

                vs = []
                for f in range(FPSB):
                    for ci, (m0, msz) in enumerate(((0, 128), (128, 68))):
                        vs.append(
                            v_proj(msz, 196 * f + m0, 1536, f"vs{f}_{ci}",
                                   f"vs{f}_{ci}_{s}", f"ps_vs{s}_{f}_{ci}")
                        )
                vt = []
                for w in range(WPSB):
                    vt.append(
                        v_proj(112, 112 * w, 1920, f"vt{w}",
                               f"vt{w}_{s}", f"ps_vt{s}_{w}")
                    )

                attnT = [
                    wpool.tile([128, SB], cdt, tag=f"attnT{i}", bufs=wb,
                               name=f"attnT{i}_{s}")
                    for i in range(6)
                ]

                for f in range(FPSB):
                    fo = 196 * f
                    for hp in range(3):
                        ps_avs = []
                        for hi in range(2):
                            h = 2 * hp + hi
                            pb = 64 * hi
                            qtile = qkvt[h // 2]
                            ktile = qkvt[6 + h // 2]
                            es = []
                            for ci, (m0, msz) in enumerate(((0, 128), (128, 68))):
                                ps_st = ppool.tile(
                                    [msz, 196], F32, tag="st", bufs=3,
                                    name=f"ps_st{s}_{f}_{h}_{ci}",
                                )
                                nc.tensor.matmul(
                                    ps_st[:],
                                    ktile[pb : pb + 64, fo + m0 : fo + m0 + msz],
                                    qtile[pb : pb + 64, fo : fo + 196],
                                    start=True,
                                    stop=True,
                                )
                                e = spool.tile(
                                    [msz, 196], cdt, tag="e", bufs=6,
                                    name=f"e{s}_{f}_{h}_{ci}",
                                )
                                nc.scalar.activation(
                                    e[:], ps_st[:], AF.Exp,
                                    bias=zeros_col[:msz, :], scale=SCALE,
                                )
                                es.append(e)
                            ps_av = ppool.tile(
                                [65, 196], F32, tag="av", bufs=3,
                                name=f"ps_sav{s}_{f}_{h}",
                            )
                            for ci in range(2):
                                nc.tensor.matmul(
                                    ps_av[:],
                                    vs[2 * f + ci][:, 65 * h : 65 * h + 65],
                                    es[ci][:],
                                    start=(ci == 0),
                                    stop=(ci == 1),
                                )
                            ps_avs.append(ps_av)
                        r = spool.tile([65, 392], cdt, tag="r", name=f"r{s}_{f}_{hp}")
                        with nc.allow_low_precision(reason="1/softmax-sum in cdt"):
                            for hi in range(2):
                                nc.vector.reciprocal(
                                    r[64:65, 196 * hi : 196 * hi + 196],
                                    ps_avs[hi][64:65, :],
                                )
                        ps_b = ppool.tile(
                            [64, 392], F32, tag="mm", bufs=2, name=f"ps_b{s}_{f}_{hp}"
                        )
                        nc.tensor.matmul(
                            ps_b[:], ones64[64:65, :], r[64:65, :],
                            start=True, stop=True,
                        )
                        rb = spool.tile([64, 392], cdt, tag="rb", name=f"rb{s}_{f}_{hp}")
                        flex_copy(rb[:], ps_b[:])
                        for hi in range(2):
                            h = 2 * hp + hi
                            cs = slice(196 * hi, 196 * hi + 196)
                            avc = spool.tile([64, 196], cdt, tag="avc", bufs=6,
                                             name=f"avc{s}_{f}_{h}")
                            flex_copy(avc[:], ps_avs[hi][0:64, :])
                            if hi == 0:
                                nc.vector.tensor_mul(
                                    attnT[h // 2][0:64, fo : fo + 196],
                                    avc[:], rb[:, cs],
                                )
                            else:
                                tmp = spool.tile(
                                    [64, 196], cdt, tag="tmp", name=f"tm{s}_{f}_{h}"
                                )
                                nc.vector.tensor_mul(
                                    tmp[:], avc[:], rb[:, cs]
                                )
                                nc.sync.dma_start(
                                    attnT[h // 2][64:128, fo : fo + 196], tmp[:]
                                )

                for w in range(WPSB):
                    wo = 112 * w
                    for hp in range(3):
                        ps_avs = []
                        for hi in range(2):
                            h = 6 + 2 * hp + hi
                            pb = 64 * hi
                            ps_st = ppool.tile(
                                [112, 112], F32, tag="st", bufs=3,
                                name=f"ps_tst{s}_{w}_{h}",
                            )
                            nc.tensor.matmul(
                                ps_st[:],
                                qkvt[6 + h // 2][pb : pb + 64, wo : wo + 112],
                                qkvt[h // 2][pb : pb + 64, wo : wo + 112],
                                start=True,
                                stop=True,
                            )
                            e = spool.tile(
                                [112, 112], cdt, tag="e", bufs=6,
                                name=f"et{s}_{w}_{h}",
                            )
                            nc.scalar.activation(
                                e[:], ps_st[:], AF.Exp,
                                bias=zeros_col[:112], scale=SCALE,
                            )
                            em = spool.tile(
                                [112, 112], cdt, tag="e", bufs=6,
                                name=f"em{s}_{w}_{h}",
                            )
                            nc.vector.tensor_mul(em[:], e[:], mask2_t[:, 0:112])
                            ps_av = ppool.tile(
                                [65, 112], F32, tag="av", bufs=3,
                                name=f"ps_tav{s}_{w}_{h}",
                            )
                            nc.tensor.matmul(
                                ps_av[:],
                                vt[w][:, 65 * (h - 6) : 65 * (h - 6) + 65],
                                em[:],
                                start=True,
                                stop=True,
                            )
                            ps_avs.append(ps_av)
                        r = spool.tile([65, 224], cdt, tag="r", name=f"rt{s}_{w}_{hp}")
                        with nc.allow_low_precision(reason="1/softmax-sum in cdt"):
                            for hi in range(2):
                                nc.vector.reciprocal(
                                    r[64:65, 112 * hi : 112 * hi + 112],
                                    ps_avs[hi][64:65, :],
                                )
                        ps_b = ppool.tile(
                            [64, 224], F32, tag="mm", bufs=2, name=f"ps_tb{s}_{w}_{hp}"
                        )
                        nc.tensor.matmul(
                            ps_b[:], ones64[64:65, :], r[64:65, :],
                            start=True, stop=True,
                        )
                        rb = spool.tile([64, 224], cdt, tag="rb", name=f"rbt{s}_{w}_{hp}")
                        flex_copy(rb[:], ps_b[:])
                        for hi in range(2):
                            h = 6 + 2 * hp + hi
                            cs = slice(112 * hi, 112 * hi + 112)
                            at = attnT[3 + (h - 6) // 2]
                            avc = spool.tile([64, 112], cdt, tag="avc", bufs=6,
                                             name=f"avct{s}_{w}_{h}")
                            flex_copy(avc[:], ps_avs[hi][0:64, :])
                            if hi == 0:
                                nc.vector.tensor_mul(
                                    at[0:64, wo : wo + 112], avc[:],
                                    rb[:, cs],
                                )
                            else:
                                tmp = spool.tile(
                                    [64, 112], cdt, tag="tmp", name=f"tmt{s}_{w}_{h}"
                                )
                                nc.vector.tensor_mul(
                                    tmp[:], avc[:], rb[:, cs]
                                )
                                nc.sync.dma_start(
                                    at[64:128, wo : wo + 112], tmp[:]
                                )

                for ec in range(6):
                    for j in range(2):
                        ps = ppool.tile([128, 392], F32, tag="mm", bufs=2, name=f"ps_o{s}_{ec}_{j}")
                        for dc in range(6):
                            nc.tensor.matmul(
                                ps[:],
                                mmcast(wp[dc][:, 128 * ec : 128 * (ec + 1)]),
                                mmcast(attnT[dc][:, 392 * j : 392 * (j + 1)]),
                                start=(dc == 0),
                                stop=(dc == 5),
                            )
                        ot = spool.tile([128, 392], F32, tag="ot", name=f"ot{s}_{ec}_{j}")
                        nc.scalar.activation(
                            ot[:], ps[:], AF.Identity,
                            bias=bias_t[:, ec : ec + 1], scale=1.0,
                        )
                        nc.sync.dma_start(
                            out_d.ap()[
                                128 * ec : 128 * (ec + 1),
                                so + 392 * j : so + 392 * (j + 1),
                            ],
                            ot[:],
                        )

    nc.compile()
    return nc


def _get_nc(compute: str):
    if compute not in _CACHE:
        _CACHE[compute] = _build(compute)
    return _CACHE[compute]


def _np_dtype(compute: str):
    if compute == "f32":
        return np.float32
    import ml_dtypes

    return ml_dtypes.bfloat16


def _prep_in_maps(x, w_qkv, w_proj, b_proj, compute=None):
    dt = _np_dtype(compute or COMPUTE)
    x = np.asarray(x, dtype=np.float32).reshape(B, N, D)
    xT = np.ascontiguousarray(x.transpose(0, 2, 1)).astype(dt)
    wqkvT = np.ascontiguousarray(np.asarray(w_qkv, np.float32).T).astype(dt)
    wprojT = np.ascontiguousarray(np.asarray(w_proj, np.float32).T).astype(dt)
    bias = np.asarray(b_proj, np.float32).reshape(D, 1)

    mask = np.zeros((112, 112), np.float32)
    for g in range(7):
        mask[16 * g : 16 * (g + 1), 16 * g : 16 * (g + 1)] = 1.0
    mask = mask.astype(dt)

    return [
        {"xt": xT[b], "wqkvT": wqkvT, "wprojT": wprojT, "bias": bias, "mask": mask}
        for b in range(B)
    ]


def _postprocess(results):
    out = np.stack([r["outT"].T for r in results])
    return np.ascontiguousarray(out.reshape(B, F, P, D)).astype(np.float32)


def kernel(x, w_qkv, w_proj, b_proj):
    nc = _get_nc(COMPUTE)
    in_maps = _prep_in_maps(x, w_qkv, w_proj, b_proj)
    res = run_bass_kernel_spmd(nc, in_maps, core_ids=list(range(B)))
    return _postprocess(res.results)
